# revision 8
# baseline (speedup 1.0000x reference)
"""Trainium2 Bass kernel: CellEncoder (gather -> segment-mean -> linear).

out = (segment_mean(chunk_features[member_idx], segment_ids, num_cells)) @ W + b

Strategy (8 NeuronCores, SPMD, no collectives):
  - Cells are split evenly across cores (6250 cells/core for 50k cells).
  - segment_ids is sorted, so each core's members form one contiguous slice;
    the host finds per-(core, 128-cell-block) member ranges via searchsorted
    and lays the member stream out in [128, S] subtile-major order, padded so
    every core shares one static program (subtile counts = max over cores).
  - On device, per 128-cell block: indirect-DMA gather the block's member
    rows from an augmented table [chunk|1] (the trailing 1 accumulates the
    segment counts), build one-hot member->cell matrices by comparing the
    (block-relative) segment ids against an iota row, and matmul-accumulate
    onehot^T @ [feats|1] into PSUM -> [128 cells, 256 sums + count].
  - Divide by max(count,1) (reciprocal * mult), PE-transpose to [feat, cell],
    and a 2-step matmul with W gives [128 cells, 512]; add bias, DMA out.
"""

import contextlib
import math
import os
import sys
from contextlib import ExitStack

for _p in ("/opt/trn_rl_repo",):
    if _p not in sys.path and os.path.isdir(_p):
        sys.path.insert(0, _p)

import numpy as np

import concourse.bass as bass
import concourse.tile as tile
from concourse import bacc, mybir
from concourse.bass import IndirectOffsetOnAxis
from concourse.masks import make_identity

P = 128
N_CORES = 8
F32 = mybir.dt.float32
I32 = mybir.dt.int32


def _plan(member_idx, segment_ids, num_cells):
    """Host-side sharding: per-core cell ranges and padded member streams."""
    C = int(num_cells)
    cpc = -(-C // N_CORES)          # cells per core
    nblk = -(-cpc // P)             # 128-cell blocks per core
    member_idx = np.asarray(member_idx, dtype=np.int32)
    segment_ids = np.asarray(segment_ids, dtype=np.int32)

    # bases[k, j] = first cell of block j on core k (clipped to C)
    bases = np.minimum(
        np.arange(N_CORES, dtype=np.int64)[:, None] * cpc
        + np.arange(nblk + 1, dtype=np.int64)[None, :] * P,
        C,
    )
    edges = np.searchsorted(segment_ids, bases.reshape(-1)).reshape(
        N_CORES, nblk + 1
    )
    cnts = np.diff(edges, axis=1)                      # members per (core, blk)
    nsub = np.maximum(-(-cnts // P), 1)                # subtiles per (core, blk)
    NSUB = nsub.max(axis=0).astype(np.int64)           # shared across cores
    offs = np.zeros(nblk + 1, np.int64)
    np.cumsum(NSUB, out=offs[1:])
    S_tot = int(offs[-1])

    midx_all = np.zeros((N_CORES, P, S_tot), np.int32)
    sid_all = np.full((N_CORES, P, S_tot), -1.0, np.float32)
    for k in range(N_CORES):
        for bi in range(nblk):
            e0, e1 = int(edges[k, bi]), int(edges[k, bi + 1])
            n = e1 - e0
            if n == 0:
                continue
            ns = int(NSUB[bi])
            bm = np.zeros(ns * P, np.int32)
            bs = np.full(ns * P, -1.0, np.float32)
            bm[:n] = member_idx[e0:e1]
            bs[:n] = (segment_ids[e0:e1] - int(bases[k, bi])).astype(np.float32)
            midx_all[k, :, offs[bi]:offs[bi + 1]] = bm.reshape(ns, P).T
            sid_all[k, :, offs[bi]:offs[bi + 1]] = bs.reshape(ns, P).T

    return dict(
        C=C, cpc=cpc, nblk=nblk,
        NSUB=[int(x) for x in NSUB], offs=[int(x) for x in offs],
        S_tot=S_tot, midx_all=midx_all, sid_all=sid_all,
    )


def _build(nchunk, D, DO, plan, nloops=1):
    """Emit the per-core Bass program (identical on all 8 cores).

    nloops > 1 wraps the block loop in a device-side For_i so one invocation
    executes the kernel body nloops times (benchmarking only).
    """
    nblk, NSUB, offs, S_tot = plan["nblk"], plan["NSUB"], plan["offs"], plan["S_tot"]
    NSUBmax = max(NSUB)
    DA = D + 1                      # feature row + count column
    KH = D // P                     # contraction halves for the final GEMM
    crows = nblk * P                # padded output rows per core

    nc = bacc.Bacc(
        "TRN2",
        debug=False,
        enable_asserts=False,
        target_bir_lowering=False,
        num_devices=N_CORES,
    )
    chunk_aug = nc.dram_tensor("chunk_aug", [nchunk, DA], F32, kind="ExternalInput")
    midx_d = nc.dram_tensor("midx", [P, S_tot], I32, kind="ExternalInput")
    sid_d = nc.dram_tensor("sid", [P, S_tot], F32, kind="ExternalInput")
    w_d = nc.dram_tensor("w", [D, DO], F32, kind="ExternalInput")
    brep_d = nc.dram_tensor("brep", [P, DO], F32, kind="ExternalInput")
    iota_d = nc.dram_tensor("iota", [P, NSUBmax * P], F32, kind="ExternalInput")
    out_d = nc.dram_tensor("out", [crows, DO], F32, kind="ExternalOutput")

    with tile.TileContext(nc) as tc, ExitStack() as ctx:
        const = ctx.enter_context(tc.tile_pool(name="const", bufs=1))
        feats_p = ctx.enter_context(tc.tile_pool(name="feats", bufs=3))
        oh_p = ctx.enter_context(tc.tile_pool(name="oh", bufs=3))
        cell_p = ctx.enter_context(tc.tile_pool(name="cell", bufs=2))
        cellT_p = ctx.enter_context(tc.tile_pool(name="cellT", bufs=2))
        outb_p = ctx.enter_context(tc.tile_pool(name="outb", bufs=3))
        small_p = ctx.enter_context(tc.tile_pool(name="small", bufs=2))
        ps_cf = ctx.enter_context(tc.tile_pool(name="ps_cf", bufs=2, space="PSUM"))
        ps_t = ctx.enter_context(tc.tile_pool(name="ps_t", bufs=2, space="PSUM"))
        ps_o = ctx.enter_context(tc.tile_pool(name="ps_o", bufs=2, space="PSUM"))

        midx_t = const.tile([P, S_tot], I32)
        nc.sync.dma_start(out=midx_t[:], in_=midx_d[:])
        sid_t = const.tile([P, S_tot], F32)
        nc.sync.dma_start(out=sid_t[:], in_=sid_d[:])
        w_t = const.tile([P, KH * DO], F32)
        for h in range(KH):
            nc.sync.dma_start(out=w_t[:, h * DO:(h + 1) * DO],
                              in_=w_d[h * P:(h + 1) * P, :])
        brep_t = const.tile([P, DO], F32)
        nc.sync.dma_start(out=brep_t[:], in_=brep_d[:])
        iota_t = const.tile([P, NSUBmax * P], F32)
        nc.sync.dma_start(out=iota_t[:], in_=iota_d[:])
        ident_t = const.tile([P, P], F32)
        make_identity(nc, ident_t[:])

        def body():
            for bi in range(nblk):
                ns = NSUB[bi]
                o0 = offs[bi]

                feats = feats_p.tile([P, NSUBmax * DA], F32, tag="feats")
                for s in range(ns):
                    # The HW DGE consumes one index per partition-contiguous
                    # output segment: gather one [128, DA] subtile per call.
                    nc.gpsimd.indirect_dma_start(
                        out=feats[:, s * DA:(s + 1) * DA],
                        out_offset=None,
                        in_=chunk_aug[:],
                        in_offset=IndirectOffsetOnAxis(
                            ap=midx_t[:, o0 + s:o0 + s + 1], axis=0
                        ),
                    )

                oh = oh_p.tile([P, NSUBmax * P], F32, tag="oh")
                nc.vector.tensor_tensor(
                    out=oh[:, : ns * P].rearrange("p (s j) -> p s j", s=ns),
                    in0=sid_t[:, o0:o0 + ns].to_broadcast([P, ns, P]),
                    in1=iota_t[:, : ns * P].rearrange("p (s j) -> p s j", s=ns),
                    op=mybir.AluOpType.is_equal,
                )

                psum = ps_cf.tile([P, DA], F32, tag="ps_cf")
                for s in range(ns):
                    nc.tensor.matmul(
                        out=psum[:],
                        lhsT=oh[:, s * P:(s + 1) * P],
                        rhs=feats[:, s * DA:(s + 1) * DA],
                        start=(s == 0),
                        stop=(s == ns - 1),
                    )

                cnt = small_p.tile([P, 1], F32, tag="cnt")
                nc.vector.tensor_scalar(
                    out=cnt[:], in0=psum[:, D:DA], scalar1=1.0, scalar2=None,
                    op0=mybir.AluOpType.max,
                )
                recip = small_p.tile([P, 1], F32, tag="recip")
                nc.vector.reciprocal(out=recip[:], in_=cnt[:])
                cell = cell_p.tile([P, D], F32, tag="cell")
                nc.vector.tensor_scalar(
                    out=cell[:], in0=psum[:, :D], scalar1=recip[:, 0:1],
                    scalar2=None, op0=mybir.AluOpType.mult,
                )

                cellT = cellT_p.tile([P, D], F32, tag="cellT")
                for h in range(KH):
                    pt = ps_t.tile([P, P], F32, tag="ps_t")
                    nc.tensor.transpose(
                        out=pt[:], in_=cell[:, h * P:(h + 1) * P],
                        identity=ident_t[:],
                    )
                    nc.scalar.mul(cellT[:, h * P:(h + 1) * P], pt[:], 1.0)

                po = ps_o.tile([P, DO], F32, tag="ps_o")
                for h in range(KH):
                    nc.tensor.matmul(
                        out=po[:],
                        lhsT=cellT[:, h * P:(h + 1) * P],
                        rhs=w_t[:, h * DO:(h + 1) * DO],
                        start=(h == 0),
                        stop=(h == KH - 1),
                    )

                ob = outb_p.tile([P, DO], F32, tag="ob")
                nc.vector.tensor_tensor(
                    out=ob[:], in0=po[:], in1=brep_t[:], op=mybir.AluOpType.add
                )
                nc.sync.dma_start(out=out_d[bi * P:(bi + 1) * P, :], in_=ob[:])

        if nloops > 1:
            with tc.For_i(0, nloops, 1):
                body()
        else:
            body()

    nc.compile()
    return nc


def _make_inputs(chunk_features, W, b, plan):
    nchunk, D = chunk_features.shape
    DO = W.shape[1]
    NSUBmax = max(plan["NSUB"])
    chunk_aug = np.ascontiguousarray(
        np.concatenate(
            [np.asarray(chunk_features, np.float32),
             np.ones((nchunk, 1), np.float32)],
            axis=1,
        )
    )
    iota = np.ascontiguousarray(
        np.tile(np.arange(P, dtype=np.float32), (P, NSUBmax))
    )
    brep = np.ascontiguousarray(
        np.broadcast_to(np.asarray(b, np.float32), (P, DO))
    )
    w = np.ascontiguousarray(np.asarray(W, np.float32))
    in_maps = []
    for k in range(N_CORES):
        in_maps.append({
            "chunk_aug": chunk_aug,
            "midx": np.ascontiguousarray(plan["midx_all"][k]),
            "sid": np.ascontiguousarray(plan["sid_all"][k]),
            "w": w,
            "brep": brep,
            "iota": iota,
        })
    return in_maps


def _gather_output(results, plan, DO):
    C, cpc = plan["C"], plan["cpc"]
    out = np.empty((C, DO), np.float32)
    for k in range(N_CORES):
        r0 = k * cpc
        r1 = min(C, r0 + cpc)
        out[r0:r1] = results[k]["out"][: r1 - r0]
    return out


def _run(inputs, simulate=False, trace=False):
    chunk_features = np.asarray(inputs["chunk_features"], np.float32)
    member_idx = np.asarray(inputs["member_idx"], np.int32)
    segment_ids = np.asarray(inputs["segment_ids"], np.int32)
    num_cells = int(inputs["num_cells"])
    W = np.asarray(inputs["W"], np.float32)
    b = np.asarray(inputs["b"], np.float32)
    nchunk, D = chunk_features.shape
    DO = W.shape[1]

    plan = _plan(member_idx, segment_ids, num_cells)
    nc = _build(nchunk, D, DO, plan)
    in_maps = _make_inputs(chunk_features, W, b, plan)

    if simulate:
        from concourse.bass_interp import CoreSim

        results = []
        for k in range(N_CORES):
            sim = CoreSim(nc, trace=False)
            for name, val in in_maps[k].items():
                sim.tensor(name)[:] = val
            sim.simulate()
            results.append({"out": np.array(sim.tensor("out"))})
        return _gather_output(results, plan, DO), None

    from concourse.bass_utils import run_bass_kernel_spmd

    res = run_bass_kernel_spmd(
        nc, in_maps, list(range(N_CORES)), trace=trace
    )
    return _gather_output(res.results, plan, DO), res


def kernel(**inputs):
    out, _ = _run(inputs)
    return out


# ---------------------------------------------------------------------------
# Benchmarking helpers (not used by the grading entry point).
# ---------------------------------------------------------------------------

def _make_runner(nc):
    """Replicate bass2jax.run_bass_via_pjrt's multi-core path, but split
    device_put (once) from execution (timed repeatedly)."""
    import jax
    from jax.sharding import Mesh, PartitionSpec, NamedSharding
    from jax.experimental.shard_map import shard_map
    from concourse import bass2jax, mybir as mb

    bass2jax.install_neuronx_cc_hook()
    partition_name = nc.partition_id_tensor.name if nc.partition_id_tensor else None

    in_names, out_names, out_avals, zero_outs = [], [], [], []
    for alloc in nc.m.functions[0].allocations:
        if not isinstance(alloc, mb.MemoryLocationSet):
            continue
        name = alloc.memorylocations[0].name
        if alloc.kind == "ExternalInput":
            if name != partition_name:
                in_names.append(name)
        elif alloc.kind == "ExternalOutput":
            shape = tuple(alloc.tensor_shape)
            dtype = mb.dt.np(alloc.dtype)
            out_names.append(name)
            out_avals.append(jax.core.ShapedArray(shape, dtype))
            zero_outs.append(np.zeros(shape, dtype))
    n_params = len(in_names)
    n_outs = len(out_avals)
    all_in_names = list(in_names) + list(out_names)
    if partition_name is not None:
        all_in_names.append(partition_name)
    donate = tuple(range(n_params, n_params + n_outs))

    def _body(*args):
        operands = list(args)
        if partition_name is not None:
            operands.append(bass2jax.partition_id_tensor())
        outs = bass2jax._bass_exec_p.bind(
            *operands,
            out_avals=tuple(out_avals),
            in_names=tuple(all_in_names),
            out_names=tuple(out_names),
            lowering_input_output_aliases=(),
            sim_require_finite=True,
            sim_require_nnan=True,
            nc=nc,
        )
        return tuple(outs)

    devices = jax.devices()[:N_CORES]
    mesh = Mesh(np.asarray(devices), ("core",))
    in_specs = (PartitionSpec("core"),) * (n_params + n_outs)
    out_specs = (PartitionSpec("core"),) * len(out_names)
    sharded = jax.jit(
        shard_map(_body, mesh=mesh, in_specs=in_specs, out_specs=out_specs,
                  check_rep=False),
        donate_argnums=donate,
        keep_unused=True,
    )
    sharding = NamedSharding(mesh, PartitionSpec("core"))

    def put_inputs(in_maps):
        concat_in = [
            np.concatenate([np.asarray(in_maps[c][nm]) for c in range(N_CORES)],
                           axis=0)
            for nm in in_names
        ]
        return [jax.device_put(a, sharding) for a in concat_in]

    import jax.numpy as jnp

    zeros_fn = jax.jit(
        lambda: tuple(
            jnp.zeros((N_CORES * z.shape[0], *z.shape[1:]), z.dtype)
            for z in zero_outs
        ),
        out_shardings=tuple(sharding for _ in zero_outs),
    )

    def run(dev_in):
        zeros = zeros_fn()
        outs = sharded(*dev_in, *zeros)
        jax.block_until_ready(outs)
        return outs

    return put_inputs, run, out_names, out_avals


def _bench(inputs, nloops=128, reps=8):
    """Estimate per-invocation HW time via For_i loop differencing."""
    import time

    chunk_features = np.asarray(inputs["chunk_features"], np.float32)
    member_idx = np.asarray(inputs["member_idx"], np.int32)
    segment_ids = np.asarray(inputs["segment_ids"], np.int32)
    num_cells = int(inputs["num_cells"])
    W = np.asarray(inputs["W"], np.float32)
    b = np.asarray(inputs["b"], np.float32)
    nchunk, D = chunk_features.shape
    DO = W.shape[1]

    plan = _plan(member_idx, segment_ids, num_cells)
    in_maps = _make_inputs(chunk_features, W, b, plan)

    timings = {}
    for tag, nl in (("one", 1), ("loop", nloops)):
        nc = _build(nchunk, D, DO, plan, nloops=nl)
        put_inputs, run, _, _ = _make_runner(nc)
        dev_in = put_inputs(in_maps)
        ts = []
        for r in range(reps + 1):
            t0 = time.perf_counter()
            outs = run(dev_in)
            t1 = time.perf_counter()
            ts.append(t1 - t0)
        timings[tag] = ts
        print(f"nloops={nl}: walls = {['%.4f' % t for t in ts]}")

    t1 = min(timings["one"][1:])
    tn = min(timings["loop"][1:])
    per_iter = (tn - t1) / (nloops - 1)
    print(f"estimated HW time per invocation: {per_iter * 1e9:.0f} ns")
    return per_iter


if __name__ == "__main__":
    import jax
    import reference

    with jax.default_device(jax.devices("cpu")[0]):
        inputs = reference.setup_inputs()
        inputs = {k: (np.asarray(v) if hasattr(v, "shape") else v)
                  for k, v in inputs.items()}
    _bench(inputs)
